# revision 1
# baseline (speedup 1.0000x reference)
"""Trainium2 Bass kernel for nn_LinearAttentionBlock (linear attention).

Per-core (data-parallel over batch, 1 batch / core):
  x_b [4096, 512] -> qkv = x_b @ w_qkv -> per-head LayerNorm(q), LayerNorm(k)
  dots_h = LN(k)_h^T @ v_h   [64, 64]
  out_h  = LN(q)_h @ dots_h / 4096
  out    = concat_h(out_h)   [4096, 512]

Key design:
  - Weights are column-centered per head on device, so q/k come out of the
    qkv matmul already mean-subtracted (LayerNorm mean folded into weights).
  - Variance is then just a segmented sum of squares; rstd applied with one
    stride-0-broadcast multiply per tensor (gamma/beta folded into the tiny
    per-head dots fixup instead).
  - Matmuls in bf16 with fp32 PSUM accumulation.
  - x-tile and LN(q) transposes via HWDGE DMA-transpose (xbar).
  - dots accumulated in one PSUM bank (4 head-pair blocks side by side,
    single accumulation group), out = pair-blockdiag matmul of q-hat^T.
"""
import threading

import numpy as np

import concourse.bacc as bacc
import concourse.bass as bass
import concourse.mybir as mybir
from concourse.tile import TileContext
from concourse.tile_rust import add_dep_helper

P = 128
NTOK = 4096          # tokens per batch (64*64)
CIN = 512            # input channels
N3 = 3 * CIN         # qkv columns
MT = NTOK // P       # 32 m-tiles
KC = CIN // P        # 4 k-chunks
H = 8                # heads
D = 64               # dim per head
NPAIR = H // 2       # 4 head pairs
CH = 4               # m-tiles per DMA chunk
NCORES = 8
LN_EPS = 1e-5

f32 = mybir.dt.float32
bf16 = mybir.dt.bfloat16
X = mybir.AxisListType.X
MUL = mybir.AluOpType.mult
SUB = mybir.AluOpType.subtract
ADD = mybir.AluOpType.add


def _bc(ap, n):
    """Append a stride-0 broadcast dim of size n to an AP."""
    return bass.AP(ap.tensor, ap.offset, list(ap.ap) + [[0, n]])


def _col64(dram_ap):
    """View a [64] DRAM tensor as a [64, 1] column AP (partition-major)."""
    return bass.AP(dram_ap.tensor, dram_ap.offset, [[1, D], [1, 1]])


def _body(nc, tc, pools, x, w, gq, bq, gk, bk, out):
    singles, xch, xTp, sqp, stp, kvp, outp = pools

    # ---------------- P0: weight prep ----------------
    w_f32 = singles.tile([P, KC, N3], f32)
    nc.sync.dma_start(out=w_f32[:], in_=w.rearrange("(c p) n -> p c n", p=P))

    wbar = singles.tile([P, KC, 2, H], f32)
    for part in (0, 1):
        nc.vector.reduce_sum(
            wbar[:, :, part, :],
            w_f32[:, :, part * CIN:(part + 1) * CIN].rearrange(
                "p c (h d) -> p c h d", d=D),
            axis=X)
    nc.vector.tensor_scalar_mul(out=wbar[:], in0=wbar[:], scalar1=1.0 / D)

    w_bf = singles.tile([P, KC, N3], bf16)
    for part in (0, 1):
        nc.vector.tensor_tensor(
            out=w_bf[:, :, part * CIN:(part + 1) * CIN].rearrange(
                "p c (h d) -> p c h d", d=D),
            in0=w_f32[:, :, part * CIN:(part + 1) * CIN].rearrange(
                "p c (h d) -> p c h d", d=D),
            in1=_bc(wbar[:, :, part, :], D),
            op=SUB)
    nc.vector.tensor_copy(out=w_bf[:, :, 2 * CIN:], in_=w_f32[:, :, 2 * CIN:])

    # gamma/beta columns replicated onto both partition halves
    gq2 = singles.tile([P, 1], f32)
    gk2 = singles.tile([P, 1], f32)
    bk2 = singles.tile([P, 1], f32)
    for half in (0, 1):
        sl = slice(half * D, (half + 1) * D)
        nc.sync.dma_start(out=gq2[sl, :], in_=_col64(gq))
        nc.sync.dma_start(out=gk2[sl, :], in_=_col64(gk))
        nc.sync.dma_start(out=bk2[sl, :], in_=_col64(bk))
    bq_bf = singles.tile([D, 1], bf16)
    nc.gpsimd.dma_start(out=bq_bf[:], in_=_col64(bq))

    eps_t = singles.tile([P, 1], f32)
    nc.vector.memset(eps_t[:], float(D) * LN_EPS)
    ones_bf = singles.tile([P, P], bf16)
    nc.vector.memset(ones_bf[:], 1.0)

    qhat_store = singles.tile([P, MT, CIN], bf16)
    qhatT = singles.tile([P, KC, NTOK], bf16)

    with tc.tile_pool(name="ps_acc", bufs=1, space="PSUM") as ps_acc:
        dots_ps = ps_acc.tile([P, 4 * P], f32)
        sumv_ps = ps_acc.tile([P, CIN], f32)
        with tc.tile_pool(name="ps_qkv", bufs=2, space="PSUM") as ps_qkv:
            _p1_loop(nc, x, w_bf, eps_t, ones_bf, qhat_store, qhatT,
                     dots_ps, sumv_ps,
                     (xch, xTp, sqp, stp, kvp, ps_qkv))

        # ---------------- P2: dots fixups ----------------
        dots_sb = singles.tile([P, 4 * P], f32)
        nc.vector.tensor_copy(out=dots_sb[:], in_=dots_ps[:])
        sumv_sb = singles.tile([P, CIN], f32)
        nc.vector.tensor_copy(out=sumv_sb[:], in_=sumv_ps[:])

    ktmp = singles.tile([P, NPAIR, D], f32)
    bsum = singles.tile([P, NPAIR, D], f32)
    deo = singles.tile([P, NPAIR, D], f32)
    for half in (0, 1):
        sl = slice(half * D, (half + 1) * D)
        # KV diag block, scaled by gamma_k * 8
        nc.vector.tensor_scalar(
            out=ktmp[sl, :, :],
            in0=dots_sb[sl, :].rearrange("p (pr x) -> p pr x", x=P)[
                :, :, half * D:(half + 1) * D],
            scalar1=gk2[sl, :], scalar2=8.0, op0=MUL, op1=MUL)
        # beta_k (x) sumV
        nc.vector.tensor_scalar(
            out=bsum[sl, :, :],
            in0=sumv_sb[sl, :].rearrange(
                "p (pr two d) -> p pr two d", two=2, d=D)[:, :, half, :],
            scalar1=bk2[sl, :], scalar2=None, op0=MUL)
    nc.vector.tensor_add(deo[:], ktmp[:], bsum[:])

    d_all = singles.tile([P, NPAIR, P], bf16)
    nc.vector.memset(d_all[:], 0.0)
    for half in (0, 1):
        sl = slice(half * D, (half + 1) * D)
        nc.vector.tensor_scalar(
            out=d_all[sl, :, half * D:(half + 1) * D],
            in0=deo[sl, :, :],
            scalar1=gq2[sl, :], scalar2=8.0 / NTOK, op0=MUL, op1=MUL)

    # c row: beta_q @ dots / NTOK, replicated over partitions
    dstack = singles.tile([D, H, D], bf16)
    nc.vector.tensor_copy(
        out=dstack.rearrange("p (pr two) d -> p pr two d", two=2)[:, :, 0, :],
        in_=deo[0:D, :, :])
    nc.gpsimd.dma_start(
        out=dstack.rearrange("p (pr two) d -> p pr two d", two=2)[:, :, 1, :],
        in_=deo[D:P, :, :])

    with tc.tile_pool(name="ps_fix", bufs=1, space="PSUM") as ps_fix, \
         tc.tile_pool(name="ps_out", bufs=2, space="PSUM") as ps_out:
        c_ps = ps_fix.tile([1, CIN], f32)
        nc.tensor.matmul(c_ps[:], lhsT=bq_bf[:],
                         rhs=dstack.rearrange("p h d -> p (h d)"),
                         start=True, stop=True)
        c_bf = singles.tile([1, CIN], bf16)
        nc.vector.tensor_scalar_mul(out=c_bf[:], in0=c_ps[:],
                                    scalar1=1.0 / NTOK)
        crep_ps = ps_fix.tile([P, CIN], f32)
        nc.tensor.matmul(crep_ps[:], lhsT=ones_bf[0:1, :], rhs=c_bf[:],
                         start=True, stop=True)
        crep = singles.tile([P, CIN], f32)
        nc.vector.tensor_copy(out=crep[:], in_=crep_ps[:])

        # ------------ P3: out = qhat @ D (pair blockdiag) + c ------------
        for ci in range(MT // CH):
            out_ch = outp.tile([P, CH, CIN], f32)
            for tt in range(CH):
                nt = ci * CH + tt
                o_ps = ps_out.tile([P, CIN], f32, tag="o")
                mm0 = None
                for pr in range(NPAIR):
                    mm = nc.tensor.matmul(
                        o_ps[:, pr * P:(pr + 1) * P],
                        lhsT=qhatT[:, pr, nt * P:(nt + 1) * P],
                        rhs=d_all[:, pr, :],
                        start=(pr == 0), stop=(pr == NPAIR - 1))
                    if pr == 0:
                        mm0 = mm
                    else:
                        add_dep_helper(mm.ins, mm0.ins, sync=False,
                                       reason="psum group start order")
                nc.vector.tensor_tensor(
                    out=out_ch[:, tt, :], in0=o_ps[:], in1=crep[:], op=ADD)
            nc.sync.dma_start(
                out=out[ci * CH * P:(ci + 1) * CH * P, :].rearrange(
                    "(t p) k -> p t k", p=P),
                in_=out_ch[:])


def _p1_loop(nc, x, w_bf, eps_t, ones_bf, qhat_store, qhatT,
             dots_ps, sumv_ps, pools):
    xch, xTp, sqp, stp, kvp, ps_qkv = pools
    for ci in range(MT // CH):
        x_ch = xch.tile([P, CH, CIN], bf16)
        nc.gpsimd.dma_start(
            out=x_ch[:],
            in_=x[ci * CH * P:(ci + 1) * CH * P, :].rearrange(
                "(t p) k -> p t k", p=P))
        for tt in range(CH):
            mt = ci * CH + tt
            xT = xTp.tile([P, KC, P], bf16)
            nc.sync.dma_start(out=xT[:], in_=x_ch[:, tt, :], transpose=True)

            q_ps = ps_qkv.tile([P, CIN], f32, tag="q")
            k_ps = ps_qkv.tile([P, CIN], f32, tag="k")
            v_ps = ps_qkv.tile([P, CIN], f32, tag="v")
            for nb, pst in enumerate((q_ps, k_ps, v_ps)):
                for c in range(KC):
                    nc.tensor.matmul(
                        pst[:], lhsT=xT[:, c, :],
                        rhs=w_bf[:, c, nb * CIN:(nb + 1) * CIN],
                        start=(c == 0), stop=(c == KC - 1))

            # LN stats: squares (ACT), segmented sums (DVE)
            sq_q = sqp.tile([P, CIN], f32, tag="sq_q")
            sq_k = sqp.tile([P, CIN], f32, tag="sq_k")
            nc.scalar.square(sq_q[:], q_ps[:])
            nc.scalar.square(sq_k[:], k_ps[:])
            st = stp.tile([P, 2, H], f32, tag="st")
            nc.vector.reduce_sum(
                st[:, 0, :], sq_q.rearrange("p (h d) -> p h d", d=D), axis=X)
            nc.vector.reduce_sum(
                st[:, 1, :], sq_k.rearrange("p (h d) -> p h d", d=D), axis=X)
            rstd = stp.tile([P, 2, H], f32, tag="rstd")
            nc.scalar.activation(
                out=rstd[:], in_=st[:],
                func=mybir.ActivationFunctionType.Sqrt,
                bias=eps_t[:], scale=1.0)
            nc.vector.reciprocal(rstd[:], rstd[:])

            # apply rstd (x8 factor folded into D fixup)
            nc.vector.tensor_tensor(
                out=qhat_store[:, mt, :].rearrange("p (h d) -> p h d", d=D),
                in0=q_ps.rearrange("p (h d) -> p h d", d=D),
                in1=_bc(rstd[:, 0, :], D), op=MUL)
            khat = kvp.tile([P, CIN], bf16, tag="khat")
            nc.vector.tensor_tensor(
                out=khat.rearrange("p (h d) -> p h d", d=D),
                in0=k_ps.rearrange("p (h d) -> p h d", d=D),
                in1=_bc(rstd[:, 1, :], D), op=MUL)
            v_bf = kvp.tile([P, CIN], bf16, tag="v_bf")
            nc.scalar.copy(v_bf[:], v_ps[:])

            # stage 2: dots (4 pair blocks in one bank) + sumV
            mm0 = None
            for pr in range(NPAIR):
                mm = nc.tensor.matmul(
                    dots_ps[:, pr * P:(pr + 1) * P],
                    lhsT=khat[:, pr * P:(pr + 1) * P],
                    rhs=v_bf[:, pr * P:(pr + 1) * P],
                    start=(mt == 0 and pr == 0),
                    stop=(mt == MT - 1 and pr == NPAIR - 1))
                if mt == 0:
                    if pr == 0:
                        mm0 = mm
                    else:
                        add_dep_helper(mm.ins, mm0.ins, sync=False,
                                       reason="psum group start order")
            nc.tensor.matmul(sumv_ps[:], lhsT=ones_bf[:], rhs=v_bf[:],
                             start=(mt == 0), stop=(mt == MT - 1))

            # q-hat transpose into [c, n] layout
            nc.sync.dma_start(
                out=qhatT[:, :, mt * P:(mt + 1) * P],
                in_=qhat_store[:, mt, :], transpose=True)



def build_kernel():
    nc = bacc.Bacc(None, target_bir_lowering=False)
    x = nc.declare_dram_parameter("x", [NTOK, CIN], f32, isOutput=False)[:, :]
    w = nc.declare_dram_parameter("w_qkv", [CIN, N3], f32, isOutput=False)[:, :]
    gq = nc.declare_dram_parameter("q_gamma", [D], f32, isOutput=False)[:]
    bq = nc.declare_dram_parameter("q_beta", [D], f32, isOutput=False)[:]
    gk = nc.declare_dram_parameter("k_gamma", [D], f32, isOutput=False)[:]
    bk = nc.declare_dram_parameter("k_beta", [D], f32, isOutput=False)[:]
    out = nc.declare_dram_parameter("out", [NTOK, CIN], f32, isOutput=True)[:, :]

    with TileContext(nc) as tc:
        with tc.tile_pool(name="singles", bufs=1) as singles, \
             tc.tile_pool(name="xch", bufs=2) as xch, \
             tc.tile_pool(name="xTp", bufs=3) as xTp, \
             tc.tile_pool(name="sqp", bufs=2) as sqp, \
             tc.tile_pool(name="stp", bufs=3) as stp, \
             tc.tile_pool(name="kvp", bufs=3) as kvp, \
             tc.tile_pool(name="outp", bufs=2) as outp:
            pools = (singles, xch, xTp, sqp, stp, kvp, outp)
            _body(nc, tc, pools, x, w, gq, bq, gk, bk, out)
    nc.compile()
    return nc


_LOCK = threading.Lock()
_CACHED = None


def _get_nc():
    global _CACHED
    with _LOCK:
        if _CACHED is None:
            _CACHED = build_kernel()
    return _CACHED


def kernel(x, w_qkv, q_gamma, q_beta, k_gamma, k_beta):
    from concourse.bass_utils import run_bass_kernel_spmd

    x = np.asarray(x, dtype=np.float32)
    w_qkv = np.asarray(w_qkv, dtype=np.float32)
    B, L, W, C = x.shape
    nc = _get_nc()
    in_maps = []
    for b in range(NCORES):
        in_maps.append({
            "x": np.ascontiguousarray(x[b].reshape(NTOK, CIN)),
            "w_qkv": w_qkv,
            "q_gamma": np.asarray(q_gamma, dtype=np.float32),
            "q_beta": np.asarray(q_beta, dtype=np.float32),
            "k_gamma": np.asarray(k_gamma, dtype=np.float32),
            "k_beta": np.asarray(k_beta, dtype=np.float32),
        })
    res = run_bass_kernel_spmd(nc, in_maps, list(range(NCORES)))
    out = np.stack([res.results[b]["out"] for b in range(NCORES)])
    return out.reshape(B, L, W, H * D).astype(np.float32)



# revision 2
# speedup vs baseline: 16.7565x; 16.7565x over previous
"""Trainium2 Bass kernel for nn_LinearAttentionBlock (linear attention), v3.

Per-core (data-parallel over batch, 1 batch / core):
  x_b [4096, 512] -> qkv = x_b @ w_qkv -> per-head LayerNorm(q), LayerNorm(k)
  dots_h = LN(k)_h^T @ v_h   [64, 64]
  out_h  = LN(q)_h @ dots_h / 4096
  out    = concat_h(out_h)   [4096, 512]

v3 design notes:
  - w loaded once as bf16 (cast DMA) chunk-by-chunk; centered per head so
    the first matmuls start early; v columns used uncentered directly.
  - batched transposes: one [128, 2048] DMA-transpose per 4-tile chunk for
    x and for qhat (on different queues: SP / ACT) to avoid sequencer
    head-of-line blocking.
  - squares in bf16 (ACT); Pool folds the two d-halves so DVE's segmented
    variance reduce reads half the elements.
  - gamma/beta vector loads issued mid-loop (only needed at fixup time).
  - beta_q row folded into the out accumulation group as a rank-1 matmul.
  - out tensor is bf16 (host upcasts); phase-3 PSUM->SBUF copies alternate
    ACT/DVE engines; output DMA at 4-tile granularity.
"""
import threading

import numpy as np

import concourse.bacc as bacc
import concourse.bass as bass
import concourse.mybir as mybir
from concourse.tile import TileContext
from concourse.tile_rust import add_dep_helper

P = 128
NTOK = 4096          # tokens per batch (64*64)
CIN = 512            # input channels
N3 = 3 * CIN         # qkv columns
MT = NTOK // P       # 32 m-tiles
KC = CIN // P        # 4 k-chunks
H = 8                # heads
D = 64               # dim per head
NPAIR = H // 2       # 4 head pairs
CH = 4               # m-tiles per DMA chunk
NCORES = 8
LN_EPS = 1e-5

f32 = mybir.dt.float32
bf16 = mybir.dt.bfloat16
X = mybir.AxisListType.X
MUL = mybir.AluOpType.mult
SUB = mybir.AluOpType.subtract
ADD = mybir.AluOpType.add


def _bc(ap, n):
    """Append a stride-0 broadcast dim of size n to an AP."""
    return bass.AP(ap.tensor, ap.offset, list(ap.ap) + [[0, n]])


def _col64(dram_ap):
    """View a [64] DRAM tensor as a [64, 1] column AP (partition-major)."""
    return bass.AP(dram_ap.tensor, dram_ap.offset, [[1, D], [1, 1]])


def _body(nc, tc, pools, x, w, gq, bq, gk, bk, out):
    singles, xch, xTp, sqp, stp, kvp, outp = pools

    # ---------------- P0: weight prep (chunked for early start) ----------
    # tile 0 of x first, alone: its DMA + transpose gate the very first
    # matmul, so keep them small and ahead of everything on the DMA bus.
    x_t0 = singles.tile([P, 1, CIN], bf16)
    nc.gpsimd.dma_start(
        out=x_t0[:], in_=x[0:P, :].rearrange("(t p) k -> p t k", p=P))
    xT_t0 = singles.tile([P, 1, KC, P], bf16)
    nc.sync.dma_start(out=xT_t0[:], in_=x_t0[:, 0, :], transpose=True)

    w_bf = singles.tile([P, KC, N3], bf16)
    wbar = singles.tile([P, KC, 2 * H], f32)
    w_c = singles.tile([P, KC, 2 * CIN], bf16)
    w_r = w.rearrange("(c p) n -> p c n", p=P)
    nc.gpsimd.dma_start(out=w_bf[:, 0:1, :], in_=w_r[:, 0:1, :])
    nc.gpsimd.dma_start(out=w_bf[:, 1:2, :], in_=w_r[:, 1:2, :])

    # rest of chunk 0 (tiles 1-3)
    x_ch0 = singles.tile([P, CH - 1, CIN], bf16)
    nc.gpsimd.dma_start(
        out=x_ch0[:],
        in_=x[P:CH * P, :].rearrange("(t p) k -> p t k", p=P))
    xT0 = singles.tile([P, CH - 1, KC, P], bf16)
    nc.sync.dma_start(out=xT0[:], in_=x_ch0[:], transpose=True)

    nc.gpsimd.dma_start(out=w_bf[:, 2:3, :], in_=w_r[:, 2:3, :])
    nc.gpsimd.dma_start(out=w_bf[:, 3:4, :], in_=w_r[:, 3:4, :])
    for cc in range(KC):
        nc.vector.reduce_sum(
            wbar[:, cc, :],
            w_bf[:, cc, 0:2 * CIN].rearrange("p (g d) -> p g d", d=D),
            axis=X)
        nc.vector.tensor_scalar_mul(
            out=wbar[:, cc, :], in0=wbar[:, cc, :], scalar1=1.0 / D)
        eng = nc.vector if cc % 2 == 0 else nc.gpsimd
        eng.tensor_tensor(
            out=w_c[:, cc, :].rearrange("p (g d) -> p g d", d=D),
            in0=w_bf[:, cc, 0:2 * CIN].rearrange("p (g d) -> p g d", d=D),
            in1=_bc(wbar[:, cc, :], D),
            op=SUB)

    eps_t = singles.tile([P, 1], f32)
    nc.vector.memset(eps_t[:], float(D) * LN_EPS)
    ones1 = singles.tile([1, P], bf16)
    nc.vector.memset(ones1[:], 1.0)
    ones2 = singles.tile([D + 1, P], bf16)
    nc.vector.memset(ones2[0:1, :], 1.0)
    nc.vector.memset(ones2[D:D + 1, :], 1.0)
    ones_bf = singles.tile([P, P], bf16)
    nc.vector.memset(ones_bf[:], 1.0)

    # gamma/beta staging (issued late, consumed at fixup)
    gq2 = singles.tile([P, 1], f32)
    gk2 = singles.tile([P, 1], f32)
    bk2 = singles.tile([P, 1], f32)
    bq2 = singles.tile([P, 1], f32)
    bq_bf = singles.tile([P, 1], bf16)

    qhat_store = singles.tile([P, MT, CIN], bf16)
    qhatT = singles.tile([P, MT, KC, P], bf16)

    with tc.tile_pool(name="ps_acc", bufs=1, space="PSUM") as ps_acc:
        dots_ps = ps_acc.tile([P, 4 * P], f32)
        sumv_ps = ps_acc.tile([P, CIN], f32)
        with tc.tile_pool(name="ps_qkv", bufs=2, space="PSUM") as ps_qkv:
            _p1_loop(nc, x, w_bf, w_c, eps_t, ones_bf, qhat_store, qhatT,
                     dots_ps, sumv_ps,
                     (xch, xTp, sqp, stp, kvp, ps_qkv),
                     (x_ch0, xT_t0, xT0),
                     (gq2, gk2, bk2, bq2, bq_bf, gq, gk, bk, bq))

        # ---------------- P2: dots fixups (PSUM read directly) --------
        ktmp = singles.tile([P, NPAIR, D], f32)
        bsum = singles.tile([P, NPAIR, D], f32)
        deo = singles.tile([P, NPAIR, D], bf16)
        for half in (0, 1):
            sl = slice(half * D, (half + 1) * D)
            # KV diag block, scaled by gamma_k * 8
            nc.vector.tensor_scalar(
                out=ktmp[sl, :, :],
                in0=dots_ps[sl, :].rearrange("p (pr x) -> p pr x", x=P)[
                    :, :, half * D:(half + 1) * D],
                scalar1=gk2[sl, :], scalar2=8.0, op0=MUL, op1=MUL)
            # beta_k (x) sumV
            nc.vector.tensor_scalar(
                out=bsum[sl, :, :],
                in0=sumv_ps[sl, :].rearrange(
                    "p (pr two d) -> p pr two d", two=2, d=D)[:, :, half, :],
                scalar1=bk2[sl, :], scalar2=None, op0=MUL)
        nc.vector.tensor_add(deo[:], ktmp[:], bsum[:])

    d_all = singles.tile([P, NPAIR, P], bf16)
    nc.vector.memset(d_all[:], 0.0)
    for half in (0, 1):
        sl = slice(half * D, (half + 1) * D)
        nc.vector.tensor_scalar(
            out=d_all[sl, :, half * D:(half + 1) * D],
            in0=deo[sl, :, :],
            scalar1=gq2[sl, :], scalar2=8.0 / NTOK, op0=MUL, op1=MUL)

    # c rows: beta_q @ dots (1/NTOK folded into bq_bf).  One matmul with a
    # two-column beta (one per partition half) gives c for even/odd heads
    # of each pair as two rows; the out-group adds them via two strided
    # rank-1 matmuls.
    bq_two = singles.tile([P, D + 1], bf16)
    nc.vector.memset(bq_two[:], 0.0)
    nc.vector.tensor_copy(out=bq_two[0:D, 0:1], in_=bq_bf[0:D, :])
    nc.vector.tensor_copy(out=bq_two[D:P, D:D + 1], in_=bq_bf[D:P, :])
    with tc.tile_pool(name="ps_fix", bufs=1, space="PSUM") as ps_fix:
        # rows land on partitions 0 and 64 (matmul operand base partitions
        # are restricted to 0/32/64)
        c2_ps = ps_fix.tile([D + 1, NPAIR * D], f32)
        nc.tensor.matmul(c2_ps[:], lhsT=bq_two[:],
                         rhs=deo.rearrange("p pr d -> p (pr d)"),
                         start=True, stop=True)
        c2_bf = singles.tile([D + 1, NPAIR * D], bf16)
        nc.scalar.copy(c2_bf[0:1, :], c2_ps[0:1, :])
        nc.scalar.copy(c2_bf[D:D + 1, :], c2_ps[D:D + 1, :])
        # pack the two head-half rows into one h-ordered row for the
        # single rank-1 add in the out groups (small SBUF->SBUF DMAs)
        c_bf = singles.tile([1, CIN], bf16)
        c_v = c_bf.rearrange("o (pr two d) -> o pr two d", two=2, d=D)
        nc.sync.dma_start(out=c_v[:, :, 0, :], in_=c2_bf[0:1, :])
        nc.sync.dma_start(out=c_v[:, :, 1, :], in_=c2_bf[D:D + 1, :])

    with tc.tile_pool(name="ps_out", bufs=8, space="PSUM") as ps_out:
        # ------------ P3: out = qhat @ D (pair blockdiag) + 1 (x) c ------
        for nt2 in range(MT // 2):
            out_t = outp.tile([P, 2, CIN], bf16)
            for half2 in range(2):
                nt = nt2 * 2 + half2
                o_t = ps_out.tile([P, CIN], f32, tag="o")
                mm0 = None
                for pr in range(NPAIR):
                    mm = nc.tensor.matmul(
                        o_t[:, pr * P:(pr + 1) * P],
                        lhsT=qhatT[:, nt, pr, :],
                        rhs=d_all[:, pr, :],
                        start=(pr == 0), stop=False)
                    if pr == 0:
                        mm0 = mm
                    else:
                        add_dep_helper(mm.ins, mm0.ins, sync=False,
                                       reason="psum group start order")
                mm = nc.tensor.matmul(
                    o_t[:], lhsT=ones1[:], rhs=c_bf[:],
                    start=False, stop=True)
                add_dep_helper(mm.ins, mm0.ins, sync=False,
                               reason="psum group start order")
                if half2 == 0:
                    nc.scalar.copy(out_t[:, 0, :], o_t[:])
                else:
                    nc.vector.tensor_copy(out=out_t[:, 1, :], in_=o_t[:])
            nc.sync.dma_start(
                out=out[nt2 * 2 * P:(nt2 + 1) * 2 * P, :].rearrange(
                    "(t p) k -> p t k", p=P),
                in_=out_t[:])


def _p1_loop(nc, x, w_bf, w_c, eps_t, ones_bf, qhat_store, qhatT,
             dots_ps, sumv_ps, pools, first_tiles, gb):
    xch, xTp, sqp, stp, kvp, ps_qkv = pools
    x_ch0, xT_t0, xT0 = first_tiles
    gq2, gk2, bk2, bq2, bq_bf, gq, gk, bk, bq = gb
    xch_tiles = {0: (None, (xT_t0, xT0))}
    xT_dmas = {}
    last_fold = {}

    def _prefetch(cj):
        # issue chunk cj's load + transpose ahead of the consuming chunk.
        # The ordering edge keeps the Pool queue clear: without it the
        # scheduler hoists this DMA (long buffer wait) ahead of the
        # latency-critical sq folds, head-blocking them.
        x_c = xch.tile([P, CH, CIN], bf16, name=f"x_ch{cj}", tag="x")
        xd = nc.gpsimd.dma_start(
            out=x_c[:],
            in_=x[cj * CH * P:(cj + 1) * CH * P, :].rearrange(
                "(t p) k -> p t k", p=P))
        if cj - 3 in last_fold:
            add_dep_helper(xd.ins, last_fold[cj - 3].ins, sync=False,
                           reason="Pool order: prefetch behind folds")
        xT_c = xTp.tile([P, CH, KC, P], bf16, name=f"xT{cj}", tag="xT")
        xT_dmas[cj] = nc.sync.dma_start(out=xT_c[:], in_=x_c[:],
                                        transpose=True)
        xch_tiles[cj] = (x_c, xT_c)

    _prefetch(1)
    _prefetch(2)
    add_dep_helper(xch_tiles[2][0].tensor.def_instruction().ins
                   if False else xT_dmas[2].ins, xT_dmas[1].ins, sync=False,
                   reason="SP order: xT2 after xT1")
    for ci in range(MT // CH):
        _, xT = xch_tiles.pop(ci)

        def _xT(tt):
            if ci == 0:
                return xT_t0[:, 0] if tt == 0 else xT[1][:, tt - 1]
            return xT[:, tt]  # [P, KC, P]
        if 2 <= ci <= 5:
            # small vector loads, needed only at fixup time; issued
            # mid-loop on SP (idle during the loop), ordered behind this
            # chunk's x transpose so they can't jump the startup queue
            src, dst = [(gq, gq2), (gk, gk2), (bk, bk2), (bq, bq2)][ci - 2]
            for half in (0, 1):
                sl = slice(half * D, (half + 1) * D)
                gd = nc.sync.dma_start(out=dst[sl, :], in_=_col64(src))
                if ci in xT_dmas:
                    add_dep_helper(gd.ins, xT_dmas[ci].ins, sync=False,
                                   reason="order: gamma loads late")
        if ci == 6:
            nc.vector.tensor_scalar_mul(out=bq_bf[:], in0=bq2[:],
                                        scalar1=1.0 / NTOK)
        # v only needs the uncentered weights; at startup (ci==0) emit
        # both chunk-0 v groups ahead of any q/k so PE has work while the
        # centering chain finishes.
        v_tiles = {}

        def _v_mms(tt):
            v_ps = ps_qkv.tile([P, CIN], f32, tag="v", name=f"v_ps{ci}_{tt}")
            xTt = _xT(tt)
            for c in range(KC):
                nc.tensor.matmul(
                    v_ps[:], lhsT=xTt[:, c, :], rhs=w_bf[:, c, 2 * CIN:],
                    start=(c == 0), stop=(c == KC - 1))
            v_tiles[tt] = v_ps

        if ci == 0:
            _v_mms(0)
            _v_mms(1)
        for tt in range(CH):
            mt = ci * CH + tt

            if tt not in v_tiles:
                _v_mms(tt)
            v_ps = v_tiles.pop(tt)
            q_ps = ps_qkv.tile([P, CIN], f32, tag="q")
            k_ps = ps_qkv.tile([P, CIN], f32, tag="k")
            xTt = _xT(tt)
            for c in range(KC):
                nc.tensor.matmul(
                    q_ps[:], lhsT=xTt[:, c, :], rhs=w_c[:, c, 0:CIN],
                    start=(c == 0), stop=(c == KC - 1))
            for c in range(KC):
                nc.tensor.matmul(
                    k_ps[:], lhsT=xTt[:, c, :], rhs=w_c[:, c, CIN:2 * CIN],
                    start=(c == 0), stop=(c == KC - 1))

            # Early PSUM->SBUF bf16 copies on ACT free the banks at once,
            # so the stats chain below has no PSUM-recycle deadline.
            qk_bf = sqp.tile([P, 2, CIN], bf16, tag="qk_bf")
            nc.scalar.copy(qk_bf[:, 0, :], q_ps[:])
            nc.scalar.copy(qk_bf[:, 1, :], k_ps[:])
            v_bf = kvp.tile([P, CIN], bf16, tag="v_bf")
            nc.scalar.copy(v_bf[:], v_ps[:])

            # LN stats from SBUF: squares (DVE 2x), fold halves (Pool),
            # segmented sum (DVE)
            sq2 = sqp.tile([P, 2, CIN], bf16, tag="sq2")
            nc.vector.tensor_tensor(out=sq2[:], in0=qk_bf[:], in1=qk_bf[:],
                                    op=MUL)
            sqf = sqp.tile([P, 2, H, D // 2], bf16, tag="sqf")
            fold = nc.gpsimd.tensor_tensor(
                out=sqf[:],
                in0=sq2.rearrange("p g (h e d) -> p g h e d", e=2,
                                  d=D // 2)[:, :, :, 0, :],
                in1=sq2.rearrange("p g (h e d) -> p g h e d", e=2,
                                  d=D // 2)[:, :, :, 1, :],
                op=ADD)
            if tt == CH - 1:
                last_fold[ci] = fold
            st = stp.tile([P, 2, H], f32, tag="st")
            nc.vector.reduce_sum(st[:], sqf[:], axis=X)
            rstd = stp.tile([P, 2, H], f32, tag="rstd")
            nc.scalar.activation(
                out=rstd[:], in_=st[:],
                func=mybir.ActivationFunctionType.Sqrt,
                bias=eps_t[:], scale=1.0)
            nc.vector.reciprocal(rstd[:], rstd[:])

            # apply rstd (x8 factor folded into D fixup)
            nc.vector.tensor_tensor(
                out=qhat_store[:, mt, :].rearrange("p (h d) -> p h d", d=D),
                in0=qk_bf[:, 0, :].rearrange("p (h d) -> p h d", d=D),
                in1=_bc(rstd[:, 0, :], D), op=MUL)
            khat = kvp.tile([P, CIN], bf16, tag="khat")
            nc.vector.tensor_tensor(
                out=khat.rearrange("p (h d) -> p h d", d=D),
                in0=qk_bf[:, 1, :].rearrange("p (h d) -> p h d", d=D),
                in1=_bc(rstd[:, 1, :], D), op=MUL)

            # stage 2: dots (4 pair blocks in one bank) + sumV
            mm0 = None
            for pr in range(NPAIR):
                mm = nc.tensor.matmul(
                    dots_ps[:, pr * P:(pr + 1) * P],
                    lhsT=khat[:, pr * P:(pr + 1) * P],
                    rhs=v_bf[:, pr * P:(pr + 1) * P],
                    start=(mt == 0 and pr == 0),
                    stop=(mt == MT - 1 and pr == NPAIR - 1))
                if mt == 0:
                    if pr == 0:
                        mm0 = mm
                    else:
                        add_dep_helper(mm.ins, mm0.ins, sync=False,
                                       reason="psum group start order")
            nc.tensor.matmul(sumv_ps[:], lhsT=ones_bf[:], rhs=v_bf[:],
                             start=(mt == 0), stop=(mt == MT - 1))

        if ci + 3 < MT // CH:
            _prefetch(ci + 3)
        # q-hat transposes, one batched DMA per chunk on SP.  An explicit
        # ordering edge keeps each one BEHIND the x transpose two chunks
        # ahead in SP's in-order stream: qhat tiles are produced late, and
        # SP head-blocking on them would stall the x-transpose prefetches
        # (buffer-release semaphores ride SP's stream).
        qd = nc.sync.dma_start(
            out=qhatT[:, ci * CH:(ci + 1) * CH, :, :],
            in_=qhat_store[:, ci * CH:(ci + 1) * CH, :], transpose=True)
        lookahead = ci + 2
        if lookahead in xT_dmas:
            add_dep_helper(qd.ins, xT_dmas[lookahead].ins, sync=False,
                           reason="SP order: qT behind xT prefetch")


def build_kernel():
    nc = bacc.Bacc(None, target_bir_lowering=False)
    x = nc.declare_dram_parameter("x", [NTOK, CIN], f32, isOutput=False)[:, :]
    w = nc.declare_dram_parameter("w_qkv", [CIN, N3], f32, isOutput=False)[:, :]
    gq = nc.declare_dram_parameter("q_gamma", [D], f32, isOutput=False)[:]
    bq = nc.declare_dram_parameter("q_beta", [D], f32, isOutput=False)[:]
    gk = nc.declare_dram_parameter("k_gamma", [D], f32, isOutput=False)[:]
    bk = nc.declare_dram_parameter("k_beta", [D], f32, isOutput=False)[:]
    out = nc.declare_dram_parameter("out", [NTOK, CIN], bf16, isOutput=True)[:, :]

    with TileContext(nc) as tc:
        with tc.tile_pool(name="singles", bufs=1) as singles, \
             tc.tile_pool(name="xch", bufs=3) as xch, \
             tc.tile_pool(name="xTp", bufs=3) as xTp, \
             tc.tile_pool(name="sqp", bufs=4) as sqp, \
             tc.tile_pool(name="stp", bufs=6) as stp, \
             tc.tile_pool(name="kvp", bufs=4) as kvp, \
             tc.tile_pool(name="outp", bufs=6) as outp:
            pools = (singles, xch, xTp, sqp, stp, kvp, outp)
            _body(nc, tc, pools, x, w, gq, bq, gk, bk, out)
    nc.compile()
    return nc


_LOCK = threading.Lock()
_CACHED = None


def _get_nc():
    global _CACHED
    with _LOCK:
        if _CACHED is None:
            _CACHED = build_kernel()
    return _CACHED


def kernel(x, w_qkv, q_gamma, q_beta, k_gamma, k_beta):
    from concourse.bass_utils import run_bass_kernel_spmd

    x = np.asarray(x, dtype=np.float32)
    w_qkv = np.asarray(w_qkv, dtype=np.float32)
    B, L, W, C = x.shape
    nc = _get_nc()
    in_maps = []
    for b in range(NCORES):
        in_maps.append({
            "x": np.ascontiguousarray(x[b].reshape(NTOK, CIN)),
            "w_qkv": w_qkv,
            "q_gamma": np.asarray(q_gamma, dtype=np.float32),
            "q_beta": np.asarray(q_beta, dtype=np.float32),
            "k_gamma": np.asarray(k_gamma, dtype=np.float32),
            "k_beta": np.asarray(k_beta, dtype=np.float32),
        })
    res = run_bass_kernel_spmd(nc, in_maps, list(range(NCORES)))
    out = np.stack([np.asarray(res.results[b]["out"]).astype(np.float32)
                    for b in range(NCORES)])
    return out.reshape(B, L, W, H * D)


# revision 3
# speedup vs baseline: 16.7993x; 1.0026x over previous
"""Trainium2 Bass kernel for nn_LinearAttentionBlock (linear attention), v3.

Per-core (data-parallel over batch, 1 batch / core):
  x_b [4096, 512] -> qkv = x_b @ w_qkv -> per-head LayerNorm(q), LayerNorm(k)
  dots_h = LN(k)_h^T @ v_h   [64, 64]
  out_h  = LN(q)_h @ dots_h / 4096
  out    = concat_h(out_h)   [4096, 512]

v3 design notes:
  - w loaded once as bf16 (cast DMA) chunk-by-chunk; centered per head so
    the first matmuls start early; v columns used uncentered directly.
  - batched transposes: one [128, 2048] DMA-transpose per 4-tile chunk for
    x and for qhat (on different queues: SP / ACT) to avoid sequencer
    head-of-line blocking.
  - squares in bf16 (ACT); Pool folds the two d-halves so DVE's segmented
    variance reduce reads half the elements.
  - gamma/beta vector loads issued mid-loop (only needed at fixup time).
  - beta_q row folded into the out accumulation group as a rank-1 matmul.
  - out tensor is bf16 (host upcasts); phase-3 PSUM->SBUF copies alternate
    ACT/DVE engines; output DMA at 4-tile granularity.
"""
import threading

import numpy as np

import concourse.bacc as bacc
import concourse.bass as bass
import concourse.mybir as mybir
from concourse.tile import TileContext
from concourse.tile_rust import add_dep_helper

P = 128
NTOK = 4096          # tokens per batch (64*64)
CIN = 512            # input channels
N3 = 3 * CIN         # qkv columns
MT = NTOK // P       # 32 m-tiles
KC = CIN // P        # 4 k-chunks
H = 8                # heads
D = 64               # dim per head
NPAIR = H // 2       # 4 head pairs
CH = 4               # m-tiles per DMA chunk
NCORES = 8
LN_EPS = 1e-5

f32 = mybir.dt.float32
bf16 = mybir.dt.bfloat16
X = mybir.AxisListType.X
MUL = mybir.AluOpType.mult
SUB = mybir.AluOpType.subtract
ADD = mybir.AluOpType.add


def _bc(ap, n):
    """Append a stride-0 broadcast dim of size n to an AP."""
    return bass.AP(ap.tensor, ap.offset, list(ap.ap) + [[0, n]])


def _col64(dram_ap):
    """View a [64] DRAM tensor as a [64, 1] column AP (partition-major)."""
    return bass.AP(dram_ap.tensor, dram_ap.offset, [[1, D], [1, 1]])


def _body(nc, tc, pools, x, w, gq, bq, gk, bk, out):
    singles, xch, xTp, sqp, stp, kvp, outp = pools

    # ---------------- P0: weight prep (chunked for early start) ----------
    # tile 0 of x first, alone: its DMA + transpose gate the very first
    # matmul, so keep them small and ahead of everything on the DMA bus.
    x_t0 = singles.tile([P, 1, CIN], bf16)
    nc.gpsimd.dma_start(
        out=x_t0[:], in_=x[0:P, :].rearrange("(t p) k -> p t k", p=P))
    xT_t0 = singles.tile([P, 1, KC, P], bf16)
    nc.sync.dma_start(out=xT_t0[:], in_=x_t0[:, 0, :], transpose=True)

    w_bf = singles.tile([P, KC, N3], bf16)
    wbar = singles.tile([P, KC, 2 * H], f32)
    w_c = singles.tile([P, KC, 2 * CIN], bf16)
    w_r = w.rearrange("(c p) n -> p c n", p=P)
    nc.gpsimd.dma_start(out=w_bf[:, 0:1, :], in_=w_r[:, 0:1, :])
    nc.gpsimd.dma_start(out=w_bf[:, 1:2, :], in_=w_r[:, 1:2, :])

    # rest of chunk 0 (tiles 1-3)
    x_ch0 = singles.tile([P, CH - 1, CIN], bf16)
    nc.gpsimd.dma_start(
        out=x_ch0[:],
        in_=x[P:CH * P, :].rearrange("(t p) k -> p t k", p=P))
    xT0 = singles.tile([P, CH - 1, KC, P], bf16)
    nc.sync.dma_start(out=xT0[:], in_=x_ch0[:], transpose=True)

    nc.gpsimd.dma_start(out=w_bf[:, 2:3, :], in_=w_r[:, 2:3, :])
    nc.gpsimd.dma_start(out=w_bf[:, 3:4, :], in_=w_r[:, 3:4, :])
    for cc in range(KC):
        nc.vector.reduce_sum(
            wbar[:, cc, :],
            w_bf[:, cc, 0:2 * CIN].rearrange("p (g d) -> p g d", d=D),
            axis=X)
        nc.vector.tensor_scalar_mul(
            out=wbar[:, cc, :], in0=wbar[:, cc, :], scalar1=1.0 / D)
        eng = nc.vector if cc % 2 == 0 else nc.gpsimd
        eng.tensor_tensor(
            out=w_c[:, cc, :].rearrange("p (g d) -> p g d", d=D),
            in0=w_bf[:, cc, 0:2 * CIN].rearrange("p (g d) -> p g d", d=D),
            in1=_bc(wbar[:, cc, :], D),
            op=SUB)

    eps_t = singles.tile([P, 1], f32)
    nc.vector.memset(eps_t[:], float(D) * LN_EPS)
    ones1 = singles.tile([1, P], bf16)
    nc.vector.memset(ones1[:], 1.0)
    ones2 = singles.tile([D + 1, P], bf16)
    nc.vector.memset(ones2[0:1, :], 1.0)
    nc.vector.memset(ones2[D:D + 1, :], 1.0)
    ones_bf = singles.tile([P, P], bf16)
    nc.vector.memset(ones_bf[:], 1.0)

    # gamma/beta staging (issued late, consumed at fixup)
    gq2 = singles.tile([P, 1], f32)
    gk2 = singles.tile([P, 1], f32)
    bk2 = singles.tile([P, 1], f32)
    bq2 = singles.tile([P, 1], f32)
    bq_bf = singles.tile([P, 1], bf16)

    qhat_store = singles.tile([P, MT, CIN], bf16)
    qhatT = singles.tile([P, MT, KC, P], bf16)

    with tc.tile_pool(name="ps_acc", bufs=1, space="PSUM") as ps_acc:
        dots_ps = ps_acc.tile([P, 4 * P], f32)
        sumv_ps = ps_acc.tile([P, CIN], f32)
        with tc.tile_pool(name="ps_qkv", bufs=2, space="PSUM") as ps_qkv:
            _p1_loop(nc, x, w_bf, w_c, eps_t, ones_bf, qhat_store, qhatT,
                     dots_ps, sumv_ps,
                     (xch, xTp, sqp, stp, kvp, ps_qkv),
                     (x_ch0, xT_t0, xT0),
                     (gq2, gk2, bk2, bq2, bq_bf, gq, gk, bk, bq))

        # ---------------- P2: dots fixups (PSUM read directly) --------
        ktmp = singles.tile([P, NPAIR, D], f32)
        bsum = singles.tile([P, NPAIR, D], f32)
        deo = singles.tile([P, NPAIR, D], bf16)
        for half in (0, 1):
            sl = slice(half * D, (half + 1) * D)
            # KV diag block, scaled by gamma_k * 8
            nc.vector.tensor_scalar(
                out=ktmp[sl, :, :],
                in0=dots_ps[sl, :].rearrange("p (pr x) -> p pr x", x=P)[
                    :, :, half * D:(half + 1) * D],
                scalar1=gk2[sl, :], scalar2=8.0, op0=MUL, op1=MUL)
            # beta_k (x) sumV -- on ACT (per-partition scale) so it runs
            # in parallel with ktmp on DVE
            nc.scalar.activation(
                out=bsum[sl, :, :],
                in_=sumv_ps[sl, :].rearrange(
                    "p (pr two d) -> p pr two d", two=2, d=D)[:, :, half, :],
                func=mybir.ActivationFunctionType.Copy,
                scale=bk2[sl, :])
        nc.vector.tensor_add(deo[:], ktmp[:], bsum[:])

    d_all = singles.tile([P, NPAIR, P], bf16)
    nc.vector.memset(d_all[:], 0.0)
    for half in (0, 1):
        sl = slice(half * D, (half + 1) * D)
        nc.vector.tensor_scalar(
            out=d_all[sl, :, half * D:(half + 1) * D],
            in0=deo[sl, :, :],
            scalar1=gq2[sl, :], scalar2=8.0 / NTOK, op0=MUL, op1=MUL)

    # c rows: beta_q @ dots (1/NTOK folded into bq_bf).  One matmul with a
    # two-column beta (one per partition half) gives c for even/odd heads
    # of each pair as two rows; the out-group adds them via two strided
    # rank-1 matmuls.
    bq_two = singles.tile([P, D + 1], bf16)
    nc.vector.memset(bq_two[:], 0.0)
    nc.vector.tensor_copy(out=bq_two[0:D, 0:1], in_=bq_bf[0:D, :])
    nc.vector.tensor_copy(out=bq_two[D:P, D:D + 1], in_=bq_bf[D:P, :])
    with tc.tile_pool(name="ps_fix", bufs=1, space="PSUM") as ps_fix:
        # rows land on partitions 0 and 64 (matmul operand base partitions
        # are restricted to 0/32/64)
        c2_ps = ps_fix.tile([D + 1, NPAIR * D], f32)
        nc.tensor.matmul(c2_ps[:], lhsT=bq_two[:],
                         rhs=deo.rearrange("p pr d -> p (pr d)"),
                         start=True, stop=True)
        c2_bf = singles.tile([D + 1, NPAIR * D], bf16)
        nc.scalar.copy(c2_bf[0:1, :], c2_ps[0:1, :])
        nc.vector.tensor_copy(out=c2_bf[D:D + 1, :], in_=c2_ps[D:D + 1, :])
        # pack the two head-half rows into one h-ordered row (two small
        # SBUF->SBUF DMAs on separate queues so they overlap)
        c_bf = singles.tile([1, CIN], bf16)
        c_v = c_bf.rearrange("o (pr two d) -> o pr two d", two=2, d=D)
        nc.sync.dma_start(out=c_v[:, :, 0, :], in_=c2_bf[0:1, :])
        nc.scalar.dma_start(out=c_v[:, :, 1, :], in_=c2_bf[D:D + 1, :])

    with tc.tile_pool(name="ps_out", bufs=8, space="PSUM") as ps_out:
        # ------------ P3: out = qhat @ D (pair blockdiag) + 1 (x) c ------
        for nt2 in range(MT // 2):
            out_t = outp.tile([P, 2, CIN], bf16)
            for half2 in range(2):
                nt = nt2 * 2 + half2
                o_t = ps_out.tile([P, CIN], f32, tag="o")
                mm0 = None
                for pr in range(NPAIR):
                    mm = nc.tensor.matmul(
                        o_t[:, pr * P:(pr + 1) * P],
                        lhsT=qhatT[:, nt, pr, :],
                        rhs=d_all[:, pr, :],
                        start=(pr == 0), stop=False)
                    if pr == 0:
                        mm0 = mm
                    else:
                        add_dep_helper(mm.ins, mm0.ins, sync=False,
                                       reason="psum group start order")
                mm = nc.tensor.matmul(
                    o_t[:], lhsT=ones1[:], rhs=c_bf[:],
                    start=False, stop=True)
                add_dep_helper(mm.ins, mm0.ins, sync=False,
                               reason="psum group start order")
                if half2 == 0:
                    nc.scalar.copy(out_t[:, 0, :], o_t[:])
                else:
                    nc.vector.tensor_copy(out=out_t[:, 1, :], in_=o_t[:])
            nc.sync.dma_start(
                out=out[nt2 * 2 * P:(nt2 + 1) * 2 * P, :].rearrange(
                    "(t p) k -> p t k", p=P),
                in_=out_t[:])


def _p1_loop(nc, x, w_bf, w_c, eps_t, ones_bf, qhat_store, qhatT,
             dots_ps, sumv_ps, pools, first_tiles, gb):
    xch, xTp, sqp, stp, kvp, ps_qkv = pools
    x_ch0, xT_t0, xT0 = first_tiles
    gq2, gk2, bk2, bq2, bq_bf, gq, gk, bk, bq = gb
    xch_tiles = {0: (None, (xT_t0, xT0))}
    xT_dmas = {}
    last_fold = {}

    def _prefetch(cj):
        # issue chunk cj's load + transpose ahead of the consuming chunk.
        # The ordering edge keeps the Pool queue clear: without it the
        # scheduler hoists this DMA (long buffer wait) ahead of the
        # latency-critical sq folds, head-blocking them.
        x_c = xch.tile([P, CH, CIN], bf16, name=f"x_ch{cj}", tag="x")
        xd = nc.gpsimd.dma_start(
            out=x_c[:],
            in_=x[cj * CH * P:(cj + 1) * CH * P, :].rearrange(
                "(t p) k -> p t k", p=P))
        if cj - 3 in last_fold:
            add_dep_helper(xd.ins, last_fold[cj - 3].ins, sync=False,
                           reason="Pool order: prefetch behind folds")
        xT_c = xTp.tile([P, CH, KC, P], bf16, name=f"xT{cj}", tag="xT")
        xT_dmas[cj] = nc.sync.dma_start(out=xT_c[:], in_=x_c[:],
                                        transpose=True)
        xch_tiles[cj] = (x_c, xT_c)

    _prefetch(1)
    _prefetch(2)
    add_dep_helper(xch_tiles[2][0].tensor.def_instruction().ins
                   if False else xT_dmas[2].ins, xT_dmas[1].ins, sync=False,
                   reason="SP order: xT2 after xT1")
    for ci in range(MT // CH):
        _, xT = xch_tiles.pop(ci)

        def _xT(tt):
            if ci == 0:
                return xT_t0[:, 0] if tt == 0 else xT[1][:, tt - 1]
            return xT[:, tt]  # [P, KC, P]
        if 2 <= ci <= 5:
            # small vector loads, needed only at fixup time; issued
            # mid-loop on SP (idle during the loop), ordered behind this
            # chunk's x transpose so they can't jump the startup queue
            src, dst = [(gq, gq2), (gk, gk2), (bk, bk2), (bq, bq2)][ci - 2]
            for half in (0, 1):
                sl = slice(half * D, (half + 1) * D)
                gd = nc.sync.dma_start(out=dst[sl, :], in_=_col64(src))
                if ci in xT_dmas:
                    add_dep_helper(gd.ins, xT_dmas[ci].ins, sync=False,
                                   reason="order: gamma loads late")
        if ci == 6:
            nc.vector.tensor_scalar_mul(out=bq_bf[:], in0=bq2[:],
                                        scalar1=1.0 / NTOK)
        # v only needs the uncentered weights; at startup (ci==0) emit
        # both chunk-0 v groups ahead of any q/k so PE has work while the
        # centering chain finishes.
        v_tiles = {}

        def _v_mms(tt):
            v_ps = ps_qkv.tile([P, CIN], f32, tag="v", name=f"v_ps{ci}_{tt}")
            xTt = _xT(tt)
            for c in range(KC):
                nc.tensor.matmul(
                    v_ps[:], lhsT=xTt[:, c, :], rhs=w_bf[:, c, 2 * CIN:],
                    start=(c == 0), stop=(c == KC - 1))
            v_tiles[tt] = v_ps

        if ci == 0:
            _v_mms(0)
            _v_mms(1)
        for tt in range(CH):
            mt = ci * CH + tt

            if tt not in v_tiles:
                _v_mms(tt)
            v_ps = v_tiles.pop(tt)
            q_ps = ps_qkv.tile([P, CIN], f32, tag="q")
            k_ps = ps_qkv.tile([P, CIN], f32, tag="k")
            xTt = _xT(tt)
            for c in range(KC):
                nc.tensor.matmul(
                    q_ps[:], lhsT=xTt[:, c, :], rhs=w_c[:, c, 0:CIN],
                    start=(c == 0), stop=(c == KC - 1))
            for c in range(KC):
                nc.tensor.matmul(
                    k_ps[:], lhsT=xTt[:, c, :], rhs=w_c[:, c, CIN:2 * CIN],
                    start=(c == 0), stop=(c == KC - 1))

            # Early PSUM->SBUF bf16 copies on ACT free the banks at once,
            # so the stats chain below has no PSUM-recycle deadline.
            qk_bf = sqp.tile([P, 2, CIN], bf16, tag="qk_bf")
            nc.scalar.copy(qk_bf[:, 0, :], q_ps[:])
            nc.scalar.copy(qk_bf[:, 1, :], k_ps[:])
            v_bf = kvp.tile([P, CIN], bf16, tag="v_bf")
            nc.scalar.copy(v_bf[:], v_ps[:])

            # LN stats from SBUF: squares (DVE 2x), fold halves (Pool),
            # segmented sum (DVE)
            sq2 = sqp.tile([P, 2, CIN], bf16, tag="sq2")
            nc.vector.tensor_tensor(out=sq2[:], in0=qk_bf[:], in1=qk_bf[:],
                                    op=MUL)
            sqf = sqp.tile([P, 2, H, D // 2], bf16, tag="sqf")
            fold = nc.gpsimd.tensor_tensor(
                out=sqf[:],
                in0=sq2.rearrange("p g (h e d) -> p g h e d", e=2,
                                  d=D // 2)[:, :, :, 0, :],
                in1=sq2.rearrange("p g (h e d) -> p g h e d", e=2,
                                  d=D // 2)[:, :, :, 1, :],
                op=ADD)
            if tt == CH - 1:
                last_fold[ci] = fold
            st = stp.tile([P, 2, H], f32, tag="st")
            nc.vector.reduce_sum(st[:], sqf[:], axis=X)
            rstd = stp.tile([P, 2, H], f32, tag="rstd")
            nc.scalar.activation(
                out=rstd[:], in_=st[:],
                func=mybir.ActivationFunctionType.Sqrt,
                bias=eps_t[:], scale=1.0)
            nc.vector.reciprocal(rstd[:], rstd[:])

            # apply rstd (x8 factor folded into D fixup)
            nc.vector.tensor_tensor(
                out=qhat_store[:, mt, :].rearrange("p (h d) -> p h d", d=D),
                in0=qk_bf[:, 0, :].rearrange("p (h d) -> p h d", d=D),
                in1=_bc(rstd[:, 0, :], D), op=MUL)
            khat = kvp.tile([P, CIN], bf16, tag="khat")
            nc.vector.tensor_tensor(
                out=khat.rearrange("p (h d) -> p h d", d=D),
                in0=qk_bf[:, 1, :].rearrange("p (h d) -> p h d", d=D),
                in1=_bc(rstd[:, 1, :], D), op=MUL)

            # stage 2: dots (4 pair blocks in one bank) + sumV
            mm0 = None
            for pr in range(NPAIR):
                mm = nc.tensor.matmul(
                    dots_ps[:, pr * P:(pr + 1) * P],
                    lhsT=khat[:, pr * P:(pr + 1) * P],
                    rhs=v_bf[:, pr * P:(pr + 1) * P],
                    start=(mt == 0 and pr == 0),
                    stop=(mt == MT - 1 and pr == NPAIR - 1))
                if mt == 0:
                    if pr == 0:
                        mm0 = mm
                    else:
                        add_dep_helper(mm.ins, mm0.ins, sync=False,
                                       reason="psum group start order")
            nc.tensor.matmul(sumv_ps[:], lhsT=ones_bf[:], rhs=v_bf[:],
                             start=(mt == 0), stop=(mt == MT - 1))

        if ci + 3 < MT // CH:
            _prefetch(ci + 3)
        # q-hat transposes, one batched DMA per chunk on SP.  An explicit
        # ordering edge keeps each one BEHIND the x transpose two chunks
        # ahead in SP's in-order stream: qhat tiles are produced late, and
        # SP head-blocking on them would stall the x-transpose prefetches
        # (buffer-release semaphores ride SP's stream).
        qd = nc.sync.dma_start(
            out=qhatT[:, ci * CH:(ci + 1) * CH, :, :],
            in_=qhat_store[:, ci * CH:(ci + 1) * CH, :], transpose=True)
        lookahead = ci + 2
        if lookahead in xT_dmas:
            add_dep_helper(qd.ins, xT_dmas[lookahead].ins, sync=False,
                           reason="SP order: qT behind xT prefetch")


def build_kernel():
    nc = bacc.Bacc(None, target_bir_lowering=False)
    x = nc.declare_dram_parameter("x", [NTOK, CIN], f32, isOutput=False)[:, :]
    w = nc.declare_dram_parameter("w_qkv", [CIN, N3], f32, isOutput=False)[:, :]
    gq = nc.declare_dram_parameter("q_gamma", [D], f32, isOutput=False)[:]
    bq = nc.declare_dram_parameter("q_beta", [D], f32, isOutput=False)[:]
    gk = nc.declare_dram_parameter("k_gamma", [D], f32, isOutput=False)[:]
    bk = nc.declare_dram_parameter("k_beta", [D], f32, isOutput=False)[:]
    out = nc.declare_dram_parameter("out", [NTOK, CIN], bf16, isOutput=True)[:, :]

    with TileContext(nc) as tc:
        with tc.tile_pool(name="singles", bufs=1) as singles, \
             tc.tile_pool(name="xch", bufs=3) as xch, \
             tc.tile_pool(name="xTp", bufs=3) as xTp, \
             tc.tile_pool(name="sqp", bufs=4) as sqp, \
             tc.tile_pool(name="stp", bufs=6) as stp, \
             tc.tile_pool(name="kvp", bufs=4) as kvp, \
             tc.tile_pool(name="outp", bufs=6) as outp:
            pools = (singles, xch, xTp, sqp, stp, kvp, outp)
            _body(nc, tc, pools, x, w, gq, bq, gk, bk, out)
    nc.compile()
    return nc


_LOCK = threading.Lock()
_CACHED = None


def _get_nc():
    global _CACHED
    with _LOCK:
        if _CACHED is None:
            _CACHED = build_kernel()
    return _CACHED


def kernel(x, w_qkv, q_gamma, q_beta, k_gamma, k_beta):
    from concourse.bass_utils import run_bass_kernel_spmd

    x = np.asarray(x, dtype=np.float32)
    w_qkv = np.asarray(w_qkv, dtype=np.float32)
    B, L, W, C = x.shape
    nc = _get_nc()
    in_maps = []
    for b in range(NCORES):
        in_maps.append({
            "x": np.ascontiguousarray(x[b].reshape(NTOK, CIN)),
            "w_qkv": w_qkv,
            "q_gamma": np.asarray(q_gamma, dtype=np.float32),
            "q_beta": np.asarray(q_beta, dtype=np.float32),
            "k_gamma": np.asarray(k_gamma, dtype=np.float32),
            "k_beta": np.asarray(k_beta, dtype=np.float32),
        })
    res = run_bass_kernel_spmd(nc, in_maps, list(range(NCORES)))
    out = np.stack([np.asarray(res.results[b]["out"]).astype(np.float32)
                    for b in range(NCORES)])
    return out.reshape(B, L, W, H * D)
